# revision 17
# baseline (speedup 1.0000x reference)
"""PointNetPolylineEncoder Trainium2 kernel (8-core SPMD, Bass/Tile).

Strategy
--------
Data-parallel over the 8 NeuronCores: each core gets 2 of the 16 batches
(30720 point-rows).  Inside a core, rows are split into two 15360-row
"chunks" stacked on the SBUF partition axis (channels of chunk0 on
partitions 0-63, chunk1 on 64-127) so every elementwise / stats pass runs
at full 128-lane width and each matmul layer runs 2-way packed on the PE
array (tile_position (0,0) / (64,64)).

Masked BatchNorm: per-core (sum, sumsq) of each pre-activation are computed
with bn_stats directly from PSUM, corrected analytically for the masked
max-pool broadcast term, and AllReduced across the 8 cores (3 tiny
collectives).  Invalid rows are forced to exactly 0 after every ReLU by
accumulating -1e30 * invalid_indicator into the PSUM via rank-1 matmuls
(so no extra elementwise mask pass is ever needed).

The last linear layer is computed with the activation tile as the
*stationary* operand so the output lands in PSUM already row-major
([128 rows, 128 ch]), giving contiguous 512B DMA stores; the masked +b4
bias rides in an augmented contraction row (h3 row 64 = mask, W4aug row
64 = b4).
"""

import sys, os, functools

sys.path.insert(0, "/opt/trn_rl_repo")

import numpy as np

import concourse.bass as bass
import concourse.bacc as bacc
import concourse.tile as tile
from concourse import mybir
from concourse.bass_utils import run_bass_kernel_spmd

F32 = mybir.dt.float32
BF16 = mybir.dt.bfloat16

B, P, N, C = 16, 768, 20, 9
H, OUT = 64, 128
NCORES = 8
RPC = B * P * N // NCORES      # 30720 rows per core
F = RPC // 2                   # 15360 stacked columns
FP = F // N                    # wrong on purpose? no: F//20
FP = F // 20                   # 768 polylines per chunk
NT = F // 512                  # 30 tiles for 512-col layers
T1 = 480                       # L1 tile (24 polylines * 20)
NT1 = F // T1                  # 32
EPS = 1e-5
BIG = 1.0e30
SLAB = 1024                    # L4 slab (columns of one chunk)
NSLAB = F // SLAB              # 15

AX = mybir.AxisListType
ALU = mybir.AluOpType
ACTF = mybir.ActivationFunctionType

# debug knobs (affect the traced program — for bisection only)
SKIP_AR = os.environ.get("K_SKIP_AR", "0") == "1"
SKIP_POOLBC = os.environ.get("K_SKIP_POOLBC", "0") == "1"
SKIP_RANK1 = os.environ.get("K_SKIP_RANK1", "0") == "1"
STOP_AFTER = os.environ.get("K_STOP_AFTER", "")  # "", "l0", "l1", "l2"


def _build_nc():
    nc = bacc.Bacc(
        "TRN2",
        target_bir_lowering=False,
        debug=False,
        enable_asserts=True,
        num_devices=NCORES,
    )
    xfm = nc.dram_tensor("xfm", [20, F], F32, kind="ExternalInput").ap()
    mpack = nc.dram_tensor("mpack", [2, F + 64], BF16, kind="ExternalInput").ap()
    mfrow = nc.dram_tensor("mfrow", [2, F], F32, kind="ExternalInput").ap()
    wpack = nc.dram_tensor("wpack", [128, 320], F32, kind="ExternalInput").ap()
    w4aug = nc.dram_tensor("w4aug", [65, 128], F32, kind="ExternalInput").ap()
    bnp = nc.dram_tensor("bnp", [128, 16], F32, kind="ExternalInput").ap()
    i20d = nc.dram_tensor("i20", [128, FP], F32, kind="ExternalInput").ap()
    out = nc.dram_tensor("out", [RPC, OUT], F32, kind="ExternalOutput").ap()

    with tile.TileContext(nc) as tc:
        _trace(nc, tc, xfm, mpack, mfrow, wpack, w4aug, bnp, i20d, out)
    nc.compile()
    return nc


def _trace(nc, tc, xfm, mpack, mfrow, wpack, w4aug, bnp, i20d, out):
    from contextlib import ExitStack

    ctx = ExitStack()
    const = ctx.enter_context(tc.tile_pool(name="const", bufs=1))
    bigp = ctx.enter_context(tc.tile_pool(name="bigp", bufs=2))
    stp = ctx.enter_context(tc.tile_pool(name="stp", bufs=1))
    scr = ctx.enter_context(tc.tile_pool(name="scr", bufs=2))
    slabp = ctx.enter_context(tc.tile_pool(name="slabp", bufs=2))
    outp = ctx.enter_context(tc.tile_pool(name="outp", bufs=2))
    psY = ctx.enter_context(tc.tile_pool(name="psY", bufs=3, space="PSUM"))
    psO = ctx.enter_context(tc.tile_pool(name="psO", bufs=4, space="PSUM"))
    drp = ctx.enter_context(tc.tile_pool(name="drp", bufs=1, space="DRAM"))

    RG = [list(range(NCORES))]

    # ---- constants into SBUF ----
    wsb = const.tile([128, 320], F32, name="wsb")
    nc.sync.dma_start(out=wsb, in_=wpack)
    w4sb = const.tile([65, OUT], F32, name="w4sb")
    nc.sync.dma_start(out=w4sb, in_=w4aug)
    mpk = const.tile([128, F + 64], BF16, name="mpk")
    nc.sync.dma_start(out=mpk[0:1, :], in_=mpack[0:1, :])    # ind_c1 + BIGvec
    nc.sync.dma_start(out=mpk[64:65, :], in_=mpack[1:2, :])  # ind_c0 + BIGvec
    bnpt = const.tile([128, 16], F32, name="bnpt")
    nc.sync.dma_start(out=bnpt, in_=bnp)
    i20 = const.tile([128, FP], F32, name="i20")
    nc.sync.dma_start(out=i20, in_=i20d)
    epst = const.tile([128, 1], F32, name="epst")
    nc.vector.memset(epst, EPS)

    xt = bigp.tile([128, F], F32, tag="big", name="xt")
    nc.sync.dma_start(out=xt[0:10, :], in_=xfm[0:10, :])
    nc.sync.dma_start(out=xt[64:74, :], in_=xfm[10:20, :])

    # -------- helpers --------
    def aggregate_bn(parts, ntile, name):
        """parts [128, ntile, 6] bn_stats partials -> sig [128, 2] = (sum, sumsq)."""
        sig = stp.tile([128, 2], F32, tag=f"sig{name}", name=f"sig{name}")
        cm = stp.tile([128, ntile, 2], F32, tag=f"cm{name}", name=f"cm{name}")
        cmm = stp.tile([128, ntile, 2], F32, tag=f"cmm{name}", name=f"cmm{name}")
        cnts = parts[:, :, 0:6:3]
        means = parts[:, :, 1:6:3]
        m2s = parts[:, :, 2:6:3]
        nc.vector.tensor_tensor(out=cm, in0=cnts, in1=means, op=ALU.mult)
        nc.vector.reduce_sum(out=sig[:, 0:1], in_=cm, axis=AX.XY)
        nc.vector.tensor_tensor(out=cmm, in0=cm, in1=means, op=ALU.mult)
        nc.vector.reduce_sum(out=sig[:, 1:2], in_=cmm, axis=AX.XY)
        s2b = stp.tile([128, 1], F32, tag=f"s2b{name}", name=f"s2b{name}")
        nc.vector.reduce_sum(out=s2b, in_=m2s, axis=AX.XY)
        nc.vector.tensor_tensor(out=sig[:, 1:2], in0=sig[:, 1:2], in1=s2b, op=ALU.add)
        return sig

    def allreduce_sig(sig, name):
        """combine chunk halves (parts 64-127 into 0-63) then AllReduce."""
        swp = stp.tile([128, 2], F32, tag=f"swp{name}", name=f"swp{name}")
        nc.sync.dma_start(out=swp[0:64, :], in_=sig[64:128, :])
        comb = stp.tile([128, 2], F32, tag=f"comb{name}", name=f"comb{name}")
        nc.vector.tensor_tensor(
            out=comb[0:64, :], in0=sig[0:64, :], in1=swp[0:64, :], op=ALU.add
        )
        if SKIP_AR:
            return comb
        din = drp.tile([64, 2], F32, tag=f"arin{name}", name=f"arin{name}")
        dout = drp.tile([64, 2], F32, tag=f"arout{name}", name=f"arout{name}")
        nc.gpsimd.dma_start(out=din, in_=comb[0:64, :])
        nc.gpsimd.collective_compute(
            "AllReduce",
            ALU.add,
            ins=[din.opt()],
            outs=[dout.opt()],
            replica_groups=RG,
        )
        glob = stp.tile([128, 2], F32, tag=f"glob{name}", name=f"glob{name}")
        nc.gpsimd.dma_start(out=glob[0:64, :], in_=dout)
        return glob

    def compute_st(glob, bcol, gcol, becol, name):
        """global (sum,sumsq) -> st [128,2]: col0 = scale, col1 = shift."""
        u = stp.tile([128, 4], F32, tag=f"u{name}", name=f"u{name}")
        st = stp.tile([128, 2], F32, tag=f"st{name}", name=f"st{name}")
        inv = bnpt[0:64, 11:12]
        nc.vector.tensor_scalar_mul(out=u[0:64, 0:1], in0=glob[0:64, 0:1], scalar1=inv)
        nc.vector.tensor_scalar_mul(out=u[0:64, 1:2], in0=glob[0:64, 1:2], scalar1=inv)
        nc.vector.tensor_tensor(
            out=u[0:64, 2:3], in0=u[0:64, 0:1], in1=u[0:64, 0:1], op=ALU.mult
        )
        nc.vector.tensor_tensor(
            out=u[0:64, 3:4], in0=u[0:64, 1:2], in1=u[0:64, 2:3], op=ALU.subtract
        )
        nc.scalar.activation(
            out=u[0:64, 3:4], in_=u[0:64, 3:4], func=ACTF.Sqrt, bias=epst[0:64, :],
            scale=1.0,
        )
        nc.vector.reciprocal(out=u[0:64, 3:4], in_=u[0:64, 3:4])
        nc.vector.tensor_tensor(
            out=st[0:64, 0:1], in0=bnpt[0:64, gcol : gcol + 1], in1=u[0:64, 3:4],
            op=ALU.mult,
        )
        # mean' = sum/cnt + b ;  t = be - mean' * s
        nc.vector.tensor_tensor(
            out=u[0:64, 1:2], in0=u[0:64, 0:1], in1=bnpt[0:64, bcol : bcol + 1],
            op=ALU.add,
        )
        nc.vector.tensor_tensor(
            out=u[0:64, 2:3], in0=u[0:64, 1:2], in1=st[0:64, 0:1], op=ALU.mult
        )
        nc.vector.tensor_tensor(
            out=st[0:64, 1:2], in0=bnpt[0:64, becol : becol + 1], in1=u[0:64, 2:3],
            op=ALU.subtract,
        )
        nc.sync.dma_start(out=st[64:128, :], in_=st[0:64, :])
        return st

    def rank1_mask(ps, cols, width):
        """accumulate -BIG * invalid_indicator into psum tile (both halves)."""
        if SKIP_RANK1:
            return
        nc.tensor.matmul(
            out=ps[0:64, 0:width],
            lhsT=mpk[64:65, F : F + 64],
            rhs=mpk[64:65, cols],
            start=False, stop=True,
            tile_position=(64, 0),
        )
        nc.tensor.matmul(
            out=ps[64:128, 0:width],
            lhsT=mpk[0:1, F : F + 64],
            rhs=mpk[0:1, cols],
            start=False, stop=True,
            tile_position=(0, 64),
            skip_group_check=True,
        )

    def layer_mm(ps, wcol, rhs_t, cols, width, start, stop, k=64):
        """2-way stacked matmul: chunk0 at (0,0), chunk1 at (64,64)."""
        nc.tensor.matmul(
            out=ps[0:64, 0:width],
            lhsT=wsb[0:k, wcol : wcol + 64],
            rhs=rhs_t[0:k, cols],
            start=start, stop=stop,
            tile_position=(0, 0),
        )
        nc.tensor.matmul(
            out=ps[64:128, 0:width],
            lhsT=wsb[64 : 64 + k, wcol : wcol + 64],
            rhs=rhs_t[64 : 64 + k, cols],
            start=start, stop=stop,
            tile_position=(64, 64),
            skip_group_check=True,
        )

    # ================= L0 =================
    part0 = stp.tile([128, NT, 6], F32, name="part0")
    for i in range(NT):
        sl = slice(512 * i, 512 * i + 512)
        ps = psY.tile([128, 512], F32, tag="y", name="psl0a")
        layer_mm(ps, 0, xt, sl, 512, True, True, k=9)
        nc.vector.bn_stats(out=part0[:, i, :], in_=ps)
    sig0 = aggregate_bn(part0, NT, "l0")
    glob0 = allreduce_sig(sig0, "l0")
    st0 = compute_st(glob0, 0, 1, 2, "l0")

    ht = bigp.tile([128, F], F32, tag="big", name="ht")
    for i in range(NT):
        sl = slice(512 * i, 512 * i + 512)
        ps = psY.tile([128, 512], F32, tag="y", name="psl0b")
        layer_mm(ps, 0, xt, sl, 512, True, True, k=10)  # k=10: -BIG aug row
        nc.scalar.activation(
            out=ht[:, sl], in_=ps, func=ACTF.Relu,
            bias=st0[:, 1:2], scale=st0[:, 0:1],
        )

    def _early_out(sig):
        dummy = out[0:64, 0:2]
        nc.sync.dma_start(out=dummy, in_=sig[0:64, :])
        ctx.close()

    if STOP_AFTER == "l0":
        _early_out(st0)
        return

    # ================= maxpool + q =================
    pooled = const.tile([128, FP], F32, name="pooled")
    nc.vector.reduce_max(
        out=pooled, in_=ht.rearrange("p (g s) -> p g s", s=20), axis=AX.X
    )
    if STOP_AFTER == "poolonly":
        _early_out(pooled[:, 0:2])
        return

    # q = W1b.T @ pooled   (per-poly pooled contribution to y1)
    corr = stp.tile([128, 4], F32, name="corr")
    qwidths = [(0, 512), (512, 256)]
    for j, (q0, qw) in enumerate(qwidths):
        qs = slice(q0, q0 + qw)
        qp = psY.tile([128, 512], F32, tag="y", name="qp")
        layer_mm(qp, 128, pooled, qs, qw, True, True)
        zt = scr.tile([128, 512], F32, tag="zt", name="zt")
        nc.vector.tensor_tensor(
            out=zt[:, 0:qw], in0=qp[:, 0:qw], in1=i20[:, qs], op=ALU.mult
        )
        nc.vector.reduce_sum(out=corr[:, 2 * j : 2 * j + 1], in_=zt[:, 0:qw], axis=AX.X)
        zt2 = scr.tile([128, 512], F32, tag="zt2", name="zt2")
        nc.vector.tensor_tensor(
            out=zt2[:, 0:qw], in0=zt[:, 0:qw], in1=qp[:, 0:qw], op=ALU.mult
        )
        nc.vector.reduce_sum(
            out=corr[:, 2 * j + 1 : 2 * j + 2], in_=zt2[:, 0:qw], axis=AX.X
        )

    if STOP_AFTER in ("pool", "qmm"):
        _early_out(corr[:, 0:2])
        return

    # ================= L1 =================
    part1 = stp.tile([128, NT1, 6], F32, name="part1")
    for i in range(NT1):
        sl = slice(T1 * i, T1 * i + T1)
        pl = slice(24 * i, 24 * i + 24)
        if SKIP_POOLBC:
            prhs0, prhs1 = ht[0:64, sl], ht[64:128, sl]
        else:
            prhs0 = pooled[0:64, pl].unsqueeze(2).broadcast_to([64, 24, 20])
            prhs1 = pooled[64:128, pl].unsqueeze(2).broadcast_to([64, 24, 20])
        ps = psY.tile([128, 512], F32, tag="y", name="psl1a")
        layer_mm(ps, 64, ht, sl, T1, True, False)
        nc.tensor.matmul(
            out=ps[0:64, 0:T1], lhsT=wsb[0:64, 128:192], rhs=prhs0,
            start=False, stop=True, tile_position=(0, 0),
        )
        nc.tensor.matmul(
            out=ps[64:128, 0:T1], lhsT=wsb[64:128, 128:192], rhs=prhs1,
            start=False, stop=True, tile_position=(64, 64),
            skip_group_check=True,
        )
        nc.vector.bn_stats(out=part1[:, i, :], in_=ps[:, 0:T1])
    if STOP_AFTER == "l1mm":
        _early_out(part1[:, 0, 0:2])
        return
    sig1 = aggregate_bn(part1, NT1, "l1")
    # subtract pooled-term corrections for invalid slots
    nc.vector.tensor_tensor(
        out=corr[:, 0:1], in0=corr[:, 0:1], in1=corr[:, 2:3], op=ALU.add
    )
    nc.vector.tensor_tensor(
        out=corr[:, 1:2], in0=corr[:, 1:2], in1=corr[:, 3:4], op=ALU.add
    )
    nc.vector.tensor_tensor(
        out=sig1[:, 0:1], in0=sig1[:, 0:1], in1=corr[:, 0:1], op=ALU.subtract
    )
    nc.vector.tensor_tensor(
        out=sig1[:, 1:2], in0=sig1[:, 1:2], in1=corr[:, 1:2], op=ALU.subtract
    )
    glob1 = allreduce_sig(sig1, "l1")
    st1 = compute_st(glob1, 3, 4, 5, "l1")

    if STOP_AFTER == "l1":
        _early_out(st1)
        return

    h2a = bigp.tile([128, F], F32, tag="big", name="h2a")
    for i in range(NT1):
        sl = slice(T1 * i, T1 * i + T1)
        pl = slice(24 * i, 24 * i + 24)
        if SKIP_POOLBC:
            prhs0, prhs1 = ht[0:64, sl], ht[64:128, sl]
        else:
            prhs0 = pooled[0:64, pl].unsqueeze(2).broadcast_to([64, 24, 20])
            prhs1 = pooled[64:128, pl].unsqueeze(2).broadcast_to([64, 24, 20])
        ps = psY.tile([128, 512], F32, tag="y", name="psl1b")
        layer_mm(ps, 64, ht, sl, T1, True, False)
        nc.tensor.matmul(
            out=ps[0:64, 0:T1], lhsT=wsb[0:64, 128:192], rhs=prhs0,
            start=False, stop=False, tile_position=(0, 0),
        )
        nc.tensor.matmul(
            out=ps[64:128, 0:T1], lhsT=wsb[64:128, 128:192], rhs=prhs1,
            start=False, stop=False, tile_position=(64, 64),
            skip_group_check=True,
        )
        rank1_mask(ps, sl, T1)
        nc.scalar.activation(
            out=h2a[:, sl], in_=ps[:, 0:T1], func=ACTF.Relu,
            bias=st1[:, 1:2], scale=st1[:, 0:1],
        )

    # ================= L2 =================
    part2 = stp.tile([128, NT, 6], F32, name="part2")
    for i in range(NT):
        sl = slice(512 * i, 512 * i + 512)
        ps = psY.tile([128, 512], F32, tag="y", name="psl2a")
        layer_mm(ps, 192, h2a, sl, 512, True, True)
        nc.vector.bn_stats(out=part2[:, i, :], in_=ps)
    sig2 = aggregate_bn(part2, NT, "l2")
    glob2 = allreduce_sig(sig2, "l2")
    st2 = compute_st(glob2, 6, 7, 8, "l2")

    if STOP_AFTER == "l2":
        _early_out(st2)
        return

    h2b = bigp.tile([128, F], F32, tag="big", name="h2b")
    for i in range(NT):
        sl = slice(512 * i, 512 * i + 512)
        ps = psY.tile([128, 512], F32, tag="y", name="psl2b")
        layer_mm(ps, 192, h2a, sl, 512, True, False)
        rank1_mask(ps, sl, 512)
        nc.scalar.activation(
            out=h2b[:, sl], in_=ps, func=ACTF.Relu,
            bias=st2[:, 1:2], scale=st2[:, 0:1],
        )

    # ================= L3 + L4 slab pipeline =================
    b3c0 = bnpt[0:64, 9:10]
    b3c1 = bnpt[64:128, 9:10]
    for s in range(NSLAB):
        h3c0 = slabp.tile([65, SLAB], F32, tag="h3c0", name="h3c0")
        h3c1 = slabp.tile([65, SLAB], F32, tag="h3c1", name="h3c1")
        stg = slabp.tile([128, SLAB], F32, tag="stg", name="stg")
        ssl = slice(SLAB * s, SLAB * s + SLAB)
        nc.gpsimd.dma_start(out=h3c0[64:65, :], in_=mfrow[0:1, ssl])
        nc.gpsimd.dma_start(out=h3c1[64:65, :], in_=mfrow[1:2, ssl])
        for half in range(2):
            i = 2 * s + half
            sl = slice(512 * i, 512 * i + 512)
            lsl = slice(512 * half, 512 * half + 512)
            ps = psY.tile([128, 512], F32, tag="y", name="psl3")
            layer_mm(ps, 256, h2b, sl, 512, True, False)
            rank1_mask(ps, sl, 512)
            nc.scalar.activation(
                out=h3c0[0:64, lsl], in_=ps[0:64, :], func=ACTF.Relu,
                bias=b3c0, scale=1.0,
            )
            nc.vector.tensor_scalar(
                out=stg[64:128, lsl], in0=ps[64:128, :],
                scalar1=b3c1, scalar2=0.0, op0=ALU.add, op1=ALU.max,
            )
            nc.sync.dma_start(out=h3c1[0:64, lsl], in_=stg[64:128, lsl])
        # L4 on the completed slab (both chunks)
        for cidx, h3t in ((0, h3c0), (1, h3c1)):
            rowbase = cidx * F + SLAB * s
            ot = outp.tile([128, SLAB], F32, tag="ot", name="ot")
            for g in range(2):  # 2 psum banks of 4 output tiles each
                po = psO.tile([128, 512], F32, tag="o", name="po")
                for t4 in range(4):
                    t = 4 * g + t4
                    nc.tensor.matmul(
                        out=po[:, 128 * t4 : 128 * t4 + 128],
                        lhsT=h3t[0:65, 128 * t : 128 * t + 128],
                        rhs=w4sb,
                        start=True, stop=True,
                        tile_position=(0, 0),
                    )
                if g == 0:
                    nc.scalar.copy(out=ot[:, 0:512], in_=po)
                else:
                    nc.vector.tensor_copy(out=ot[:, 512:1024], in_=po)
            ov = out[rowbase : rowbase + SLAB, :].rearrange("(t p) c -> p t c", p=128)
            nc.sync.dma_start(out=ov, in_=ot.rearrange("p (t c) -> p t c", c=128))

    ctx.close()


@functools.lru_cache(maxsize=1)
def _get_nc():
    return _build_nc()


def _host_prep(polylines, polylines_mask, W0, b0, g0, be0, W1, b1, g1, be1,
               W2, b2, g2, be2, W3, b3, W4, b4):
    f32 = np.float32
    mask_full = np.asarray(polylines_mask)
    cnt = max(float(mask_full.sum()), 1.0)

    # weight packs (identical for every core)
    wpack = np.zeros((128, 320), f32)
    w0aug = np.concatenate([np.asarray(W0, f32), np.full((1, 64), -BIG, f32)], axis=0)
    wpack[0:10, 0:64] = w0aug
    wpack[64:74, 0:64] = w0aug
    W1 = np.asarray(W1, f32)
    for r in (0, 64):
        wpack[r : r + 64, 64:128] = W1[0:64]
        wpack[r : r + 64, 128:192] = W1[64:128]
        wpack[r : r + 64, 192:256] = np.asarray(W2, f32)
        wpack[r : r + 64, 256:320] = np.asarray(W3, f32)
    w4aug = np.concatenate(
        [np.asarray(W4, f32), np.asarray(b4, f32)[None, :]], axis=0
    )
    bnp = np.zeros((128, 16), f32)
    for r in (0, 64):
        for j, v in enumerate((b0, g0, be0, b1, g1, be1, b2, g2, be2, b3)):
            bnp[r : r + 64, j] = np.asarray(v, f32)
    bnp[:, 10] = cnt
    bnp[:, 11] = 1.0 / cnt

    per_core = []
    rows_all = np.asarray(polylines, f32).reshape(NCORES, RPC, C)
    m_all = mask_full.reshape(NCORES, RPC).astype(f32)
    for ci in range(NCORES):
        rows = rows_all[ci] * m_all[ci][:, None]
        m = m_all[ci]
        c0, c1 = rows[:F], rows[F:]
        m0, m1 = m[:F], m[F:]
        xfm = np.zeros((20, F), f32)
        xfm[0:9] = c0.T
        xfm[9] = 1.0 - m0
        xfm[10:19] = c1.T
        xfm[19] = 1.0 - m1
        mpack = np.zeros((2, F + 64), np.float32)
        mpack[0, 0:F] = 1.0 - m1          # ind_c1 -> partition 0
        mpack[1, 0:F] = 1.0 - m0          # ind_c0 -> partition 64
        mpack[:, F : F + 64] = -BIG
        mfrow = np.stack([m0, m1]).astype(f32)
        i20 = np.zeros((128, FP), f32)
        i20[0:64, :] = (20.0 - m0.reshape(FP, 20).sum(axis=1))[None, :]
        i20[64:128, :] = (20.0 - m1.reshape(FP, 20).sum(axis=1))[None, :]
        per_core.append(
            {
                "xfm": xfm,
                "mpack": mpack.astype(np.dtype("bfloat16"))
                if False
                else _to_bf16(mpack),
                "mfrow": mfrow,
                "wpack": wpack,
                "w4aug": w4aug,
                "bnp": bnp,
                "i20": i20,
            }
        )
    return per_core


def _to_bf16(a):
    import ml_dtypes

    return a.astype(ml_dtypes.bfloat16)


TRACE = False
LAST_RESULTS = None


def kernel(**inputs):
    global LAST_RESULTS
    nc = _get_nc()
    in_maps = _host_prep(**inputs)
    res = run_bass_kernel_spmd(
        nc, in_maps, core_ids=list(range(NCORES)), trace=TRACE
    )
    LAST_RESULTS = res
    outs = [res.results[ci]["out"] for ci in range(NCORES)]
    full = np.concatenate(outs, axis=0).reshape(B, P, N, OUT)
    return full


if __name__ == "__main__":
    sys.path.insert(0, "/root/problem")
    import reference

    inputs = {k: np.asarray(v) for k, v in reference.setup_inputs().items()}
    got = kernel(**inputs)
    exp = np.asarray(reference.reference(**reference.setup_inputs()))
    err = np.abs(got - exp).max() / (np.abs(exp).max() + 1e-30)
    print("Relative error:", err)
